# revision 45
# baseline (speedup 1.0000x reference)
"""Trainium2 Bass kernel: multi-head self-attention with RoPE + sigmoid gating.

Computes, for fixed shapes B=2, S=2048, E=1024, H=16, D=64:
    qkv = x @ w_qkv ; q,k roped (concatenated-halves layout)
    att = softmax(q k^T / sqrt(D)) ; out = (att @ v * sigmoid(x @ w_gate)) @ w_out + b_out

Sharding: 8 cores = 2 (batch) x 4 (head groups of 4 heads).  Each core computes a
row-parallel partial of the output projection for its batch (its 4 heads' slice of
the E contraction); the host sums the 4 partials per batch and adds b_out.

v2: software-pipelined schedule targeting ~100% PE occupancy.
  - attention inner loop skews PV one sk-step behind scores so exp overlaps
  - exp split: most tiles exact on ScalarE, a fraction on DVE via the
    Schraudolph bit-trick (scale*log2e*128*s + magic -> int16 -> bf16 bits)
  - gate projection matmuls are interleaved into chunk 0 as PE filler;
    out-projection + denominator-broadcast of chunk c run as filler inside
    chunk c+1
  - out-projection PSUM is DMA'd straight to DRAM (no copy)
  - reciprocal broadcast via one-hot matmul in float32r (full speed, fp32 prec)
  - ScalarE does the qkv-proj PSUM->SBUF moves (DVE stays on RoPE)

All matmuls bf16 (inputs pre-cast host-side); PSUM accumulation fp32.
Softmax skips max-subtraction (scores*scale are O(+-6)); denominator rides as
the 65th "ones" column of the PV stationary operand.

Device-side layouts (per core; host preps/permutes/casts all of these):
    xT   [1024, 2048]  x[b]^T
    wqk  [1024, 512]   columns: [q_even | q_odd | k_even | k_odd], each 128 = 4 heads x 32
    wv   [1024, 256]   v columns for the 4 heads (natural order)
    wg   [1024, 256]   w_gate columns for the 4 heads' output dims
    wo   [128, 2, 1024] w_out rows for the 4 heads, as 2 pair-tiles of 128
    cs/sn [128, 2048]  cos/sin RoPE tables, rows = 4x32 freqs, cols = position
Output: out [2048, 1024] fp32 partial (no bias).
"""

import numpy as np
import ml_dtypes

B, S, E, H, D = 2, 2048, 1024, 16, 64
HC = 4            # heads per core
NCORES = 8
KT = E // 128     # 8 contraction tiles
ST = S // 128     # 16 sequence tiles
SQ = 512          # attention sq chunk
NCH = S // SQ     # 4 chunks
ROPE_THETA = 10000.0

# exp-unit engine split: (sk, g) units in this set go to DVE via the
# bit-trick; the rest run exact exp on ScalarE.  Kept away from sk 0-4 where
# the previous chunk's reciprocal occupies the DVE queue.
BT_UNITS = ({(sk, g) for sk in (5, 9, 13) for g in (0, 1)}
            | {(3, 1), (11, 1), (15, 1)})

_CACHE = {}
PE_LOG = []   # labels of matmuls in emission order (devloop diagnostics)

# Results of the most recent kernel() call, for test harnesses.
LAST_RESULTS = None


# ---------------------------------------------------------------------------
# BIR postprocess: the walrus build in this image accepts only ONE sync-wait
# command per lowered TPB instruction (Drain/NoOp/LDWEIGHTS/...).  Tile emits
# instructions with several waits; split the excess onto preceding single-wait
# NoOps on the same engine (program order preserves the blocking semantics).
# Installed by patching concourse's compile_bir_kernel in this process.
# ---------------------------------------------------------------------------

def _split_waits(bir_bytes, limit=1):
    import json as _json
    m = _json.loads(bir_bytes)
    counter = [0]

    def fix_block(instrs):
        out = []
        for ins in instrs:
            w = ins.get("sync_info", {}).get("on_wait", [])
            if len(w) > limit:
                chunks = [w[i:i + limit] for i in range(0, len(w), limit)]
                ins["sync_info"]["on_wait"] = chunks[-1]
                for ch in chunks[:-1]:
                    counter[0] += 1
                    out.append({
                        "name": f"I-waitsplit-{counter[0]}",
                        "opcode": "NoOp",
                        "engine": ins.get("engine"),
                        "ins": [],
                        "outs": [],
                        "sync_info": {"on_update": [], "on_wait": ch},
                    })
            out.append(ins)
        return out

    def walk(d):
        if isinstance(d, dict):
            for k, v in d.items():
                if k == "instructions" and isinstance(v, list):
                    d[k] = fix_block(v)
                else:
                    walk(v)
        elif isinstance(d, list):
            for v in d:
                walk(v)

    walk(m)
    return _json.dumps(m).encode()


def _install_birfix():
    if _CACHE.get("birfix"):
        return
    _CACHE["birfix"] = True
    import concourse.bass_utils as bu
    import concourse.bass2jax as b2j

    orig = bu.compile_bir_kernel

    def patched(bir_json, tmpdir, neff_name="file.neff"):
        return orig(_split_waits(bir_json), tmpdir, neff_name=neff_name)

    bu.compile_bir_kernel = patched
    b2j.compile_bir_kernel = patched


def _build_nc():
    import concourse.bass as bass
    import concourse.mybir as mybir
    from concourse.tile import TileContext

    bf = mybir.dt.bfloat16
    f32 = mybir.dt.float32
    f32r = mybir.dt.float32r
    i16 = mybir.dt.int16
    MUL = mybir.AluOpType.mult
    SUB = mybir.AluOpType.subtract
    ADD = mybir.AluOpType.add
    Act = mybir.ActivationFunctionType

    scale = float(D) ** -0.5
    # bit-trick exp constants: bf16 bits of e^(scale*s) ~ 128*(scale*log2e*s
    # + 126.94269504)
    BT_A = float(scale * np.log2(np.e) * 128.0)
    BT_B = float(126.94269504 * 128.0)

    nc = bass.Bass()
    PE_LOG.clear()

    def MM(label, *args, **kw):
        PE_LOG.append(label)
        nc.tensor.matmul(*args, **kw)

    xT_d = nc.dram_tensor("xT", (E, S), bf, kind="ExternalInput")
    wqk_d = nc.dram_tensor("wqk", (E, 4 * 128), bf, kind="ExternalInput")
    wv_d = nc.dram_tensor("wv", (E, HC * 64), bf, kind="ExternalInput")
    wg_d = nc.dram_tensor("wg", (E, HC * 64), bf, kind="ExternalInput")
    wo_d = nc.dram_tensor("wo", (128, 2, E), bf, kind="ExternalInput")
    cs_d = nc.dram_tensor("cs", (128, S), bf, kind="ExternalInput")
    sn_d = nc.dram_tensor("sn", (128, S), bf, kind="ExternalInput")
    out_d = nc.dram_tensor("out", (S, E), bf, kind="ExternalOutput")

    with TileContext(nc) as tc:
        with (
            tc.tile_pool(name="const", bufs=1) as cpool,
            tc.tile_pool(name="big", bufs=1) as bpool,
            tc.tile_pool(name="work", bufs=3) as wpool,
            tc.tile_pool(name="expool", bufs=8) as expool,
            tc.tile_pool(name="scp", bufs=2, space="PSUM") as scpool,
            tc.tile_pool(name="pvp", bufs=1, space="PSUM") as pvpool,
        ):
            # ---- constants / weights in (wqk + xT first: they gate the PE) ----
            wqk = cpool.tile([128, KT, 512], bf)
            xT = bpool.tile([128, KT, S], bf)
            for k in range(KT):
                nc.sync.dma_start(wqk[:, k, :], wqk_d[k * 128:(k + 1) * 128, :])
                nc.sync.dma_start(xT[:, k, 0:1024], xT_d[k * 128:(k + 1) * 128, 0:1024])
            for k in range(KT):
                nc.sync.dma_start(xT[:, k, 1024:S], xT_d[k * 128:(k + 1) * 128, 1024:S])
            cs = cpool.tile([128, S], bf)
            nc.sync.dma_start(cs, cs_d[:, :])
            sn = cpool.tile([128, S], bf)
            nc.sync.dma_start(sn, sn_d[:, :])
            wv = cpool.tile([128, KT, 256], bf)
            nc.sync.dma_start(wv, wv_d[:, :].rearrange("(k p) m -> p k m", p=128))
            wg = cpool.tile([128, KT, 256], bf)
            nc.sync.dma_start(wg, wg_d[:, :].rearrange("(k p) m -> p k m", p=128))
            wo = cpool.tile([128, 2, E], bf)
            nc.sync.dma_start(wo, wo_d[:, :, :])

            # warm the ACT exp/tanh table set before it is on the critical path
            warm = cpool.tile([1, 8], f32)
            nc.vector.memset(warm, 0.0)
            nc.scalar.activation(warm, warm, Act.Exp)

            # one-hot rows for the matmul-based partition broadcast of the
            # softmax reciprocals: hot[32h, 128h + r] = 1  (bf16: 0/1 exact)
            hot = cpool.tile([128, HC * 128], bf)
            nc.gpsimd.memset(hot, 0.0)
            for h in range(HC):
                nc.gpsimd.memset(hot[32 * h:32 * h + 1, 128 * h:128 * (h + 1)], 1.0)

            # ---- phase A: qk projection (4 M-tiles) + RoPE, per 1024-col slice ----
            qkraw = [bpool.tile([128, S], bf, tag=f"qkraw{m}", name=f"qkraw{m}") for m in range(4)]
            qTop = bpool.tile([128, S], bf)
            qBot = bpool.tile([128, S], bf)
            kTop = bpool.tile([128, S], bf)
            kBot = bpool.tile([128, S], bf)
            qR = [bpool.tile([128, S], bf, tag=f"qR{g}", name=f"qR{g}") for g in range(2)]
            kR = [bpool.tile([128, S], bf, tag=f"kR{g}", name=f"kR{g}") for g in range(2)]
            for n2 in range(S // 1024):
                sl = slice(n2 * 1024, (n2 + 1) * 1024)
                for m in range(4):
                    ps = scpool.tile([128, 1024], f32, tag="sc", name="ps_qk")
                    for half in range(2):
                        o = 512 * half
                        for k in range(KT):
                            MM(f"qkproj n{n2} m{m} k{k}",
                                ps[:, o:o + 512],
                                lhsT=wqk[:, k, m * 128:(m + 1) * 128],
                                rhs=xT[:, k, n2 * 1024 + o:n2 * 1024 + o + 512],
                                start=(k == 0), stop=(k == KT - 1),
                            )
                    # ScalarE moves psum -> sbuf (keeps DVE free for RoPE)
                    nc.scalar.copy(qkraw[m][:, sl], ps)
                # RoPE for this 1024-column slice (full 128-partition ops)
                for (ev, od, top, bot) in ((qkraw[0], qkraw[1], qTop, qBot),
                                           (qkraw[2], qkraw[3], kTop, kBot)):
                    t1 = wpool.tile([128, 1024], bf, tag="rt1", name="rt1")
                    t2 = wpool.tile([128, 1024], bf, tag="rt2", name="rt2")
                    nc.vector.tensor_tensor(t1, ev[:, sl], cs[:, sl], MUL)
                    nc.vector.tensor_tensor(t2, od[:, sl], sn[:, sl], MUL)
                    nc.vector.tensor_tensor(top[:, sl], t1, t2, SUB)
                    t3 = wpool.tile([128, 1024], bf, tag="rt1", name="rt3")
                    t4 = wpool.tile([128, 1024], bf, tag="rt2", name="rt4")
                    nc.vector.tensor_tensor(t3, ev[:, sl], sn[:, sl], MUL)
                    nc.vector.tensor_tensor(t4, od[:, sl], cs[:, sl], MUL)
                    nc.vector.tensor_tensor(bot[:, sl], t3, t4, ADD)
                # assemble per-pair roped tensors for this slice
                # qR[g] rows: [64*h2 + j] j<32: top of head 2g+h2 ; j>=32: bottom
                for g in range(2):
                    for (top, bot, dst) in ((qTop, qBot, qR[g]), (kTop, kBot, kR[g])):
                        for h2 in range(2):
                            h = 2 * g + h2
                            nc.sync.dma_start(dst[64 * h2:64 * h2 + 32, sl],
                                              top[32 * h:32 * h + 32, sl])
                            nc.sync.dma_start(dst[64 * h2 + 32:64 * h2 + 64, sl],
                                              bot[32 * h:32 * h + 32, sl])

            # ---- phase B: v projection into [v_h | 1] stationary tiles ----
            vOnes = []
            for s in range(ST):
                # padded to 128 cols/head ([v(64) | 1 | zeros]) so the PV
                # stationary/output use the full 128 partitions
                vo = bpool.tile([128, HC * 128], bf, tag=f"vo{s}", name=f"vo{s}")
                vOnes.append(vo)
                nc.gpsimd.memset(vo, 0.0)
                for h in range(HC):
                    nc.gpsimd.memset(vo[:, h * 128 + 64:h * 128 + 65], 1.0)
                ps = scpool.tile([128, 1024], f32, tag="sc", name="ps_v")
                for k in range(KT):
                    MM(f"vproj s{s} k{k}",
                        ps[:, :256],
                        lhsT=xT[:, k, s * 128:(s + 1) * 128],
                        rhs=wv[:, k, :],
                        start=(k == 0), stop=(k == KT - 1),
                    )
                nc.scalar.copy(
                    vo.rearrange("p (h w) -> p h w", w=128)[:, :, 0:64],
                    ps[:, :256].rearrange("p (h w) -> p h w", w=64),
                )

            # ---- attention with interleaved fillers ----
            gP = [bpool.tile([128, S], bf, tag=f"gP{g}", name=f"gP{g}") for g in range(2)]
            ag = [bpool.tile([128, S], bf, tag=f"ag{g}", name=f"ag{g}") for g in range(2)]

            def gate_group(g, half2):
                # half a gate-projection column-tile: 8 matmuls + tanh + affine
                def emit():
                    o = half2 * 512
                    sl = slice(o, o + 1024) if False else slice(o, o + 512)
                    ps = scpool.tile([128, 1024], f32, tag="sc", name="ps_g")
                    for k in range(KT):
                        MM(f"gate g{g} o{o} k{k}",
                            ps[:, 0:512],
                            lhsT=wg[:, k, g * 128:(g + 1) * 128],
                            rhs=xT[:, k, o:o + 512],
                            start=(k == 0), stop=(k == KT - 1),
                        )
                    th = wpool.tile([128, 512], bf, tag="th", name="th")
                    nc.scalar.activation(th, ps[:, 0:512], Act.Tanh, scale=0.5)
                    # sigmoid(x) = 0.5*tanh(x/2) + 0.5
                    nc.vector.tensor_scalar(gP[g][:, sl], th, 0.5, 0.5, MUL, ADD)
                return emit

            def recb_group(c, rec_bf, rec128, recBs):
                # broadcast 1/den from single partitions to head pair-tiles;
                # the final chunk skips the bf16 cast (fp32 matmul straight
                # from rec128 -- slower per row but off the critical chain)
                def emit(half=None):
                    cols = slice(0, 512) if half is None else \
                        slice(half * 256, half * 256 + 256)
                    for pair in range(2):
                        if half in (None, 0):
                            ps = scpool.tile([128, 1024], f32, tag="sc",
                                             name="ps_rb")
                            recBs[pair] = ps
                        ps = recBs[pair]
                        for h2 in range(2):
                            h = 2 * pair + h2
                            MM(f"recB c{c} h{h}",
                                ps[:, h2 * 512 + cols.start:
                                    h2 * 512 + cols.stop],
                                lhsT=hot[:, 128 * h:128 * (h + 1)],
                                rhs=rec_bf[:, cols],
                                start=True, stop=True,
                            )
                return emit

            def norm_tail_h(c, recBs, uus, h, half=0):
                # after recB: agh = uu * recB ; ag = agh * gate  (DVE + gpsimd;
                # per-head granularity so DVE bt-exp units interleave)
                def emit():
                    csl = slice(c * SQ, (c + 1) * SQ)
                    g, h2 = divmod(h, 2)
                    o = 64 * h2
                    if c == NCH - 1:
                        # uu is already gated: one mul to normalized output
                        hf = emit.half
                        cw = slice(hf * 256, hf * 256 + 256)
                        nc.vector.tensor_tensor(
                            ag[g][o:o + 64, c * SQ + cw.start:c * SQ + cw.stop],
                            uus[h][o:o + 64, cw],
                            recBs[g][o:o + 64, 512 * h2 + cw.start:
                                     512 * h2 + cw.stop], MUL)
                        return
                    agh = wpool.tile([128, SQ], f32, tag=f"agh{h}", name="agh", bufs=1)
                    nc.vector.tensor_tensor(
                        agh[o:o + 64, :], uus[h][o:o + 64, :],
                        recBs[g][o:o + 64, 512 * h2:512 * (h2 + 1)], MUL)
                    nc.gpsimd.tensor_tensor(
                        ag[g][o:o + 64, csl], agh[o:o + 64, :],
                        gP[g][o:o + 64, csl], MUL)
                emit.half = half
                return emit

            def oproj_group(c, st):
                # output projection for row-tile st of chunk c
                def emit():
                    s = (SQ // 128) * c + st
                    ps = scpool.tile([128, 1024], f32, tag="sc", name="ps_o")
                    for n in range(2):
                        for g in range(2):
                            MM(f"oproj c{c} st{st} n{n} g{g}",
                                ps[:, n * 512:(n + 1) * 512],
                                lhsT=ag[g][:, s * 128:(s + 1) * 128],
                                rhs=wo[:, g, n * 512:(n + 1) * 512],
                                start=(g == 0), stop=(g == 1),
                            )
                    ob = wpool.tile([128, 1024], bf, tag="ob", name="ob", bufs=3)
                    # final chunk: ScalarE is idle, split the copies
                    if c == NCH - 1 and st % 2 == 1:
                        nc.scalar.copy(ob, ps)
                    else:
                        nc.vector.tensor_copy(ob, ps)
                    nc.sync.dma_start(out_d[s * 128:(s + 1) * 128, :], ob)
                emit.deferred = (c == NCH - 2)
                return emit

            # filler queues: emitted between sk-steps of each chunk
            fillers = {c: [] for c in range(NCH + 1)}
            fillers[0] = [gate_group(g, h2) for h2 in range(4) for g in range(2)]

            norm_state = {}
            for c in range(NCH):
                csl = slice(c * SQ, (c + 1) * SQ)
                pv = [pvpool.tile([128, SQ], f32, tag=f"pv{h}", name=f"pv{h}")
                      for h in range(HC)]
                exs = {}

                def scores_step(sk):
                    for g in range(2):
                        sct = scpool.tile([128, 1024], f32, tag="sc", name="sct")
                        for h2 in range(2):
                            MM(f"score c{c} sk{sk} g{g} h{h2}",
                                sct[:, h2 * 512:(h2 + 1) * 512],
                                lhsT=kR[g][64 * h2:64 * (h2 + 1), sk * 128:(sk + 1) * 128],
                                rhs=qR[g][64 * h2:64 * (h2 + 1), csl],
                                start=True, stop=True,
                            )
                        ex = expool.tile([128, 1024], bf, tag="ex", name="ex")
                        if (sk, g) in BT_UNITS:
                            # Schraudolph bit-trick on DVE: int16 bits of bf16
                            nc.vector.tensor_scalar(
                                ex.bitcast(i16), sct, BT_A, BT_B, MUL, ADD)
                        else:
                            nc.scalar.activation(ex, sct, Act.Exp, scale=scale)
                        exs[(sk, g)] = ex

                def pv_step(sk):
                    for g in range(2):
                        ex = exs.pop((sk, g))
                        for h2 in range(2):
                            h = 2 * g + h2
                            MM(f"pv c{c} sk{sk} h{h}",
                                pv[h][0:128, :],
                                lhsT=vOnes[sk][:, h * 128:(h + 1) * 128],
                                rhs=ex[:, h2 * 512:(h2 + 1) * 512],
                                start=(sk == 0), stop=(sk == ST - 1),
                            )

                fq = fillers[c]
                for sk in range(ST):
                    scores_step(sk)
                    if sk == 2 and norm_state.get("recip") is not None:
                        # previous chunk's reciprocal: emitted here so it sits
                        # BEHIND this chunk's early DVE exp units in the queue
                        norm_state.pop("recip")()
                    if sk > 2:
                        pv_step(sk - 3)
                    if fq and sk >= 7:
                        fq.pop(0)()
                for sk in (ST - 3, ST - 2, ST - 1):
                    pv_step(sk)
                while fq:
                    fq.pop(0)()

                # ---- normalize: free pv banks early, recip off critical path ----
                # uu copies split DVE (h0,h1) / ACT (h2,h3) release the pv
                # banks fast; dstack runs on ACT so the DVE can go straight to
                # the reciprocal.
                dstack = wpool.tile([128, SQ], f32, tag="dstack", name="dstack", bufs=2)
                nc.gpsimd.memset(dstack, 1.0)
                # ALL normalize copies stay on DVE for non-final chunks: any
                # copy placed on ScalarE would sit in its in-order queue and
                # stall the next chunk's exp stream behind PV(15).
                last = (c == NCH - 1)
                uus = []
                for h in range(HC):
                    uu = wpool.tile([128, SQ], f32, tag=f"uu{h}", name=f"uu{h}", bufs=1)
                    g_, h2_ = divmod(h, 2)
                    o_ = 64 * h2_
                    if last:
                        # gated u: fold the gate multiply in here so it
                        # overlaps the reciprocal instead of trailing recB
                        nc.vector.tensor_tensor(
                            uu[o_:o_ + 64, :], pv[h][0:64, :],
                            gP[g_][o_:o_ + 64, csl], MUL)
                        nc.scalar.copy(dstack[32 * h:32 * h + 1, :],
                                       pv[h][64:65, :])
                    elif h % 2 == 0:
                        # fused u+den copy (rows 0:65), den lands in uu[64]
                        nc.vector.tensor_copy(uu[0:65, :], pv[h][0:65, :])
                    else:
                        nc.vector.tensor_copy(uu[64:128, :], pv[h][0:64, :])
                        nc.vector.tensor_copy(dstack[32 * h:32 * h + 1, :],
                                              pv[h][64:65, :])
                    uus.append(uu)
                # even-head denominators are in SBUF: assemble on gpsimd
                if not last:
                    for h in (0, 2):
                        nc.gpsimd.tensor_copy(dstack[32 * h:32 * h + 1, :],
                                              uus[h][64:65, :])
                rec128 = wpool.tile([128, SQ], f32, tag="rec128", name="rec128", bufs=2)
                rec_bf = wpool.tile([128, SQ], bf, tag="rec_bf", name="rec_bf", bufs=2)

                def emit_recip(c=c, dstack=dstack, rec128=rec128, rec_bf=rec_bf):
                    if c == NCH - 1:
                        # two 256-wide waves so the broadcast/normalize/oproj
                        # of wave A overlap the reciprocal of wave B
                        nc.vector.reciprocal(out=rec128[:, 0:256],
                                             in_=dstack[:, 0:256])
                        nc.vector.tensor_copy(rec_bf[:, 0:256], rec128[:, 0:256])
                        nc.vector.reciprocal(out=rec128[:, 256:512],
                                             in_=dstack[:, 256:512])
                        nc.scalar.copy(rec_bf[:, 256:512], rec128[:, 256:512])
                    else:
                        nc.vector.reciprocal(out=rec128, in_=dstack)
                        nc.gpsimd.tensor_copy(rec_bf, rec128)

                if c == NCH - 1:
                    emit_recip()
                else:
                    norm_state["recip"] = emit_recip

                recBs = [None, None]
                rg = recb_group(c, rec_bf, rec128, recBs)
                if c == NCH - 1:
                    # two waves: [recbA, agA, st0, st1] then [recbB, agB, st2, st3]
                    for half in range(2):
                        fillers[c + 1].append(lambda rg=rg, hf=half: rg(hf))
                        for h in range(HC):
                            fillers[c + 1].append(
                                norm_tail_h(c, recBs, uus, h, half))
                        for st in (2 * half, 2 * half + 1):
                            fillers[c + 1].append(oproj_group(c, st))
                else:
                    fillers[c + 1].append(rg)
                    for h in range(HC):
                        fillers[c + 1].append(norm_tail_h(c, recBs, uus, h))
                    for st in range(SQ // 128):
                        # the last chunk's predecessor defers three oproj
                        # groups into the tail to cover the normalize chain
                        dst = c + 2 if c == NCH - 2 else c + 1
                        fillers[dst].append(oproj_group(c, st))

            # drain: deferred oproj groups first (they cover the final
            # normalize chain), then the final chunk's recb/norm/oproj
            fillers[NCH].sort(key=lambda f: 0 if getattr(f, "deferred", False) else 1)
            for f in fillers[NCH]:
                f()

    return nc


def _host_inputs(x, w_qkv, w_gate, w_out):
    """Build the 8 per-core input maps (all device tensors bf16 except none)."""
    bf = ml_dtypes.bfloat16
    x = np.asarray(x, dtype=np.float32)
    w_qkv = np.asarray(w_qkv, dtype=np.float32)
    w_gate = np.asarray(w_gate, dtype=np.float32)
    w_out = np.asarray(w_out, dtype=np.float32)

    inv = 1.0 / (ROPE_THETA ** (np.arange(0, D, 2, dtype=np.float64) / D))   # [32]
    ang = np.arange(S, dtype=np.float64)[None, :] * inv[:, None]             # [32, S]
    cs = np.tile(np.cos(ang), (4, 1)).astype(bf)                             # [128, S]
    sn = np.tile(np.sin(ang), (4, 1)).astype(bf)

    wq = w_qkv[:, 0:E]
    wk = w_qkv[:, E:2 * E]
    wvv = w_qkv[:, 2 * E:3 * E]

    in_maps = []
    for c in range(NCORES):
        b = c // 4
        hs = HC * (c % 4)
        cols_ev = np.concatenate([(hs + h) * 64 + np.arange(0, 64, 2) for h in range(HC)])
        cols_od = cols_ev + 1
        wqk_p = np.concatenate(
            [wq[:, cols_ev], wq[:, cols_od], wk[:, cols_ev], wk[:, cols_od]], axis=1)
        vcols = np.concatenate([(hs + h) * 64 + np.arange(64) for h in range(HC)])
        wo_p = w_out[vcols, :].reshape(2, 128, E).transpose(1, 0, 2)
        in_maps.append({
            "xT": np.ascontiguousarray(x[b].T).astype(bf),
            "wqk": np.ascontiguousarray(wqk_p).astype(bf),
            "wv": np.ascontiguousarray(wvv[:, vcols]).astype(bf),
            "wg": np.ascontiguousarray(w_gate[:, vcols]).astype(bf),
            "wo": np.ascontiguousarray(wo_p).astype(bf),
            "cs": cs,
            "sn": sn,
        })
    return in_maps


def kernel(x, w_qkv, w_gate, w_out, b_out, n_heads):
    global LAST_RESULTS
    assert int(n_heads) == H
    x = np.asarray(x)
    assert x.shape == (B, S, E)

    from concourse.bass_utils import run_bass_kernel_spmd

    _install_birfix()
    if "nc" not in _CACHE:
        _CACHE["nc"] = _build_nc()
    nc = _CACHE["nc"]

    in_maps = _host_inputs(x, w_qkv, w_gate, w_out)
    import os
    trace = bool(int(os.environ.get("KERNEL_TRACE", "0")))
    tmpdir = os.environ.get("KERNEL_TRACE_DIR") if trace else None
    res = run_bass_kernel_spmd(nc, in_maps, list(range(NCORES)), trace=trace,
                               tmpdir=tmpdir)
    LAST_RESULTS = res

    out = np.zeros((B, S, E), dtype=np.float32)
    for c in range(NCORES):
        out[c // 4] += np.asarray(res.results[c]["out"], dtype=np.float32)
    out += np.asarray(b_out, dtype=np.float32)[None, None, :]
    return out
